# revision 7
# baseline (speedup 1.0000x reference)
"""Memory-efficient multi-head attention on 8 TRN2 NeuronCores (bf16 core).

Problem (hardcoded): B=2, L=2048, D=1024, H=16, HD=64.
  out = softmax((X_q Wq + bq)(X_k Wk + bk)^T / sqrt(HD)) (X_v Wv + bv) Wo + bo

Sharding: 8 cores = 2 batches x 4 head-groups (4 heads each).  Each core gets
its batch's activations (transposed, bf16) plus its head-group's weight
slices, and produces a partial pre-bias output outT [D, L] in fp32.  Host
sums the 4 partials per batch and adds bo.

Device kernel design (cost-model-driven):
  - All matmul operands are bf16 (1 PE cycle/row, same as fp32r, but half
    the HBM traffic and SBUF).  PSUM accumulation stays fp32.  bf16
    per-element rounding (~0.1% rms) contributes ~0.2% output error vs the
    2e-2 budget.  (fp8 DoubleRow was evaluated and measured ~9% error - the
    attention average shrinks signal as fast as noise, so per-element fp8
    error passes through at full strength.  Rejected.)
  - Scores are computed transposed, sT[kpos, q] = k . q, per head on disjoint
    64-partition groups; softmax denominator comes from an appended ones
    column on v (attnv accumulator row 64).
  - exp on ScalarE with the 1/sqrt(HD) scale folded in, no max-subtraction
    (scores ~N(0,1); softmax is shift-invariant; fp32/bf16 range is ample).
  - Biases are applied by DVE during PSUM->SBUF copy-out (tensor_scalar_add
    with per-partition bias for q/k, tensor_tensor add with a host-
    broadcast bias tile for v) - no PE bias matmuls.
  - A Pool-engine memset chain paces 6 tiny dummy matmuls over the first
    ~9us so the PE p-state ramp (cost model: warm after 3us of busy) is
    done before real matmuls arrive.
  - PE is the bottleneck (~164us busy); the emission order keeps it fed:
    prologue projections, then an ACT-paced score/exp/attnv stream with
    v-proj, q-proj(qc1) and out-proj(qc0) chunks interleaved as fillers in
    the PSUM slot gaps, out-proj(qc1) as the tail.
"""

import numpy as np
import ml_dtypes

import concourse.mybir as mybir
import concourse.tile as tile
from concourse import bacc
from concourse.bass_utils import run_bass_kernel_spmd

F32 = mybir.dt.float32
BF16 = mybir.dt.bfloat16
NPBF16 = ml_dtypes.bfloat16


def build_attention_core(L=2048, D=1024, H_LOC=4, HD=64):
    JC = H_LOC * HD                   # local head channels (256)
    NJT = JC // 128                   # j-tiles (2)
    NDT = D // 128                    # contraction tiles (8)
    NLT = L // 128                    # k-position tiles (16)
    XC = 512                          # x stream chunk width
    NXC = L // XC                     # 4
    QC = 1024                         # attention q-chunk (exp tile width)
    NQC = L // QC                     # 2
    CS = 512                          # matmul N-chunk inside a q-chunk
    NCS = QC // CS                    # 2

    nc = bacc.Bacc("TRN2", target_bir_lowering=False, debug=False, num_devices=8)

    xq = nc.dram_tensor("xq", [128, NDT, L], BF16, kind="ExternalInput").ap()
    xk = nc.dram_tensor("xk", [128, NDT, L], BF16, kind="ExternalInput").ap()
    xv = nc.dram_tensor("xv", [128, NDT, L], BF16, kind="ExternalInput").ap()
    wq = nc.dram_tensor("wq", [128, NDT, JC], BF16, kind="ExternalInput").ap()
    wk = nc.dram_tensor("wk", [128, NDT, JC], BF16, kind="ExternalInput").ap()
    wv = nc.dram_tensor("wv", [128, NDT, JC], BF16, kind="ExternalInput").ap()
    wo = nc.dram_tensor("wo", [128, NJT, D], BF16, kind="ExternalInput").ap()
    bq = nc.dram_tensor("bq", [128, NJT], F32, kind="ExternalInput").ap()
    bk = nc.dram_tensor("bk", [128, NJT], F32, kind="ExternalInput").ap()
    bv = nc.dram_tensor("bv", [128, JC], F32, kind="ExternalInput").ap()
    outT = nc.dram_tensor("outT", [D, L], F32, kind="ExternalOutput").ap()

    from contextlib import ExitStack

    with (
        tile.TileContext(nc) as tc,
        ExitStack() as stack,
        nc.allow_low_precision(reason="bf16 operands, fp32 accumulation"),
    ):
        consts = stack.enter_context(tc.tile_pool(name="consts", bufs=1))
        warm = stack.enter_context(tc.tile_pool(name="warm", bufs=1))
        wpool = stack.enter_context(tc.tile_pool(name="wpool", bufs=1))
        xpool = stack.enter_context(tc.tile_pool(name="xpool", bufs=1))
        prod = stack.enter_context(tc.tile_pool(name="prod", bufs=1))
        att_pool = stack.enter_context(tc.tile_pool(name="att", bufs=3))
        rden_pool = stack.enter_context(tc.tile_pool(name="rden", bufs=2))
        oc_pool = stack.enter_context(tc.tile_pool(name="oc", bufs=4))
        psum = stack.enter_context(tc.tile_pool(name="psum", bufs=1, space="PSUM"))

        # ---- PE warmup: Pool memset chain paces tiny matmuls ~1.5us apart
        wsrc = warm.tile([1, 8], BF16, tag="wsrc")
        nc.vector.memset(wsrc, 0.0)
        ps_wm = psum.tile([1, 8], F32, tag="st0", name="ps_wm")
        for i in range(6):
            wch = warm.tile([1, 1800], F32, tag="wch", name="wch")
            nc.gpsimd.memset(wch, 0.0)
            gate = warm.tile([1, 2], BF16, tag="wg", bufs=2, name="gate")
            nc.vector.tensor_copy(out=gate, in_=wch[:, 0:2])
            nc.tensor.matmul(ps_wm[:, 0:1], gate[:, 0:1], gate[:, 1:2],
                             start=True, stop=True)

        # ---- weights + biases (lead the DMA queue)
        wk_sb = wpool.tile([128, NDT, JC], BF16, tag="wk")
        wq_sb = wpool.tile([128, NDT, JC], BF16, tag="wq")
        wv_sb = wpool.tile([128, NDT, JC], BF16, tag="wv")
        wo_sb = wpool.tile([128, NJT, D], BF16, tag="wo")
        bq_sb = wpool.tile([128, NJT], F32, tag="bq")
        bk_sb = wpool.tile([128, NJT], F32, tag="bk")
        bv_sb = wpool.tile([128, JC], F32, tag="bv")
        nc.sync.dma_start(out=wk_sb, in_=wk)
        nc.sync.dma_start(out=bk_sb, in_=bk)
        nc.sync.dma_start(out=wq_sb, in_=wq)
        nc.sync.dma_start(out=bq_sb, in_=bq)

        # ---- x streams (SBUF-resident, chunked DMA)
        xk_sb = xpool.tile([128, NDT, L], BF16, tag="xk")
        xq_sb = xpool.tile([128, NDT, L], BF16, tag="xq")
        xv_sb = xpool.tile([128, NDT, L], BF16, tag="xv")

        def load_x(eng, x_sb, x_dram, c):
            eng.dma_start(out=x_sb[:, :, c * XC:(c + 1) * XC],
                          in_=x_dram[:, :, c * XC:(c + 1) * XC])

        load_x(nc.sync, xk_sb, xk, 0)
        load_x(nc.sync, xq_sb, xq, 0)
        load_x(nc.sync, xq_sb, xq, 1)
        load_x(nc.sync, xk_sb, xk, 1)
        load_x(nc.sync, xk_sb, xk, 2)
        load_x(nc.sync, xk_sb, xk, 3)
        # v stream + remaining q on the scalar queue (HWDGE shared anyway)
        nc.scalar.dma_start(out=wv_sb, in_=wv)
        nc.scalar.dma_start(out=bv_sb, in_=bv)
        for c in range(NXC):
            load_x(nc.scalar, xv_sb, xv, c)
        load_x(nc.scalar, xq_sb, xq, 2)
        load_x(nc.scalar, xq_sb, xq, 3)
        nc.scalar.dma_start(out=wo_sb, in_=wo)

        # ---- products
        qT_sb = prod.tile([128, NJT, L], BF16, tag="qT")    # [j%128, jt, l]
        kT_sb = prod.tile([128, NJT, L], BF16, tag="kT")
        v_sb = prod.tile([128, NLT, H_LOC, HD + 1], BF16, tag="v")
        onorm_sb = prod.tile([128, NJT, L], BF16, tag="onorm")

        vones_f32 = consts.tile([128, NLT * H_LOC], F32)
        nc.vector.memset(vones_f32, 1.0)
        nc.vector.tensor_copy(
            out=v_sb[:, :, :, HD:HD + 1].rearrange("p a b c -> p (a b c)"),
            in_=vones_f32,
        )

        # ---- emission helpers ------------------------------------------
        def proj_kq(w_sb, b_sb, dst, c, jt, tag):
            """dst[j, l] = sum_d W[d, j] xT[d, l] + b[j] for one (chunk, jt)."""
            x_sb = xk_sb if dst is kT_sb else xq_sb
            ps = psum.tile([128, XC], F32, tag=tag, name="pskq")
            for dt in range(NDT):
                nc.tensor.matmul(
                    ps,
                    w_sb[:, dt, jt * 128:(jt + 1) * 128],
                    x_sb[:, dt, c * XC:(c + 1) * XC],
                    start=(dt == 0),
                    stop=(dt == NDT - 1),
                )
            nc.vector.tensor_scalar_add(
                out=dst[:, jt, c * XC:(c + 1) * XC],
                in0=ps,
                scalar1=b_sb[:, jt:jt + 1],
            )

        def proj_v(lt, tag, copy_eng):
            """v[l, (h, hd)] = sum_d xT[d, l] W[d, j] + bv[j], one 128-row tile."""
            ps = psum.tile([128, JC], F32, tag=tag, name="psv")
            for dt in range(NDT):
                nc.tensor.matmul(
                    ps,
                    xv_sb[:, dt, lt * 128:lt * 128 + 128],
                    wv_sb[:, dt, :],
                    start=(dt == 0),
                    stop=(dt == NDT - 1),
                )
            copy_eng.tensor_tensor(
                out=v_sb[:, lt, :, 0:HD],
                in0=ps.rearrange("p (h d) -> p h d", h=H_LOC),
                in1=bv_sb.rearrange("p (h d) -> p h d", h=H_LOC),
                op=mybir.AluOpType.add,
            )

        def score_exp(hp, qc, kt, h):
            """One head's transposed scores + exp at one kt.  Returns at tile."""
            hb = (h % 2) * HD
            st = psum.tile([128, QC], F32, tag=f"st{h % 2}", name=f"st{h % 2}")
            for c in range(NCS):
                nc.tensor.matmul(
                    st[:, c * CS:(c + 1) * CS],
                    kT_sb[hb:hb + HD, hp, kt * 128:kt * 128 + 128],
                    qT_sb[hb:hb + HD, hp,
                          qc * QC + c * CS:qc * QC + (c + 1) * CS],
                    start=True,
                    stop=True,
                )
            at = att_pool.tile([128, QC], BF16, tag=f"at{h % 2}",
                               name=f"at{h % 2}")
            nc.scalar.activation(
                out=at, in_=st,
                func=mybir.ActivationFunctionType.Exp,
                scale=float(1.0 / np.sqrt(HD)),
            )
            return at

        def attnv(hp, kt, h, at, ot):
            hg = 2 * hp + (h % 2)
            for c in range(NCS):
                nc.tensor.matmul(
                    ot[:, c * CS:(c + 1) * CS],
                    v_sb[:, kt, hg, :],
                    at[:, c * CS:(c + 1) * CS],
                    start=(kt == 0),
                    stop=(kt == NLT - 1),
                )

        def epilogue(hp, qc, ots):
            """onorm[ch, q] = OT[ch, q] * recip(OT[HD, q])."""
            for h in (0, 1):
                hb = h * HD
                rden = rden_pool.tile([1, QC], F32, tag="rden", name="rden")
                nc.vector.reciprocal(out=rden, in_=ots[h][HD:HD + 1, :])
                rbc = rden_pool.tile([HD, QC], F32, tag="rbc", name="rbc")
                nc.gpsimd.partition_broadcast(rbc, rden)
                nc.vector.tensor_mul(
                    out=onorm_sb[hb:hb + HD, hp, qc * QC:(qc + 1) * QC],
                    in0=ots[h][0:HD, :],
                    in1=rbc,
                )

        _store_rr = [0]

        def outproj_mt(qc, mt, tag, copy_eng):
            """outT[dp, l] = sum_j wo[j, dp] onorm[j, l], one 128-row tile."""
            ps = psum.tile([128, QC], F32, tag=tag, name="psc")
            for c in range(NCS):
                for jt in range(NJT):
                    nc.tensor.matmul(
                        ps[:, c * CS:(c + 1) * CS],
                        wo_sb[:, jt, mt * 128:(mt + 1) * 128],
                        onorm_sb[:, jt,
                                 qc * QC + c * CS:qc * QC + (c + 1) * CS],
                        start=(jt == 0),
                        stop=(jt == NJT - 1),
                    )
            ob = oc_pool.tile([128, QC], F32, tag="oc", name="oc")
            if copy_eng is nc.scalar:
                nc.scalar.copy(out=ob, in_=ps)
            else:
                copy_eng.tensor_copy(out=ob, in_=ps)
            dma_eng = (nc.sync, nc.scalar)[_store_rr[0] % 2]
            _store_rr[0] += 1
            dma_eng.dma_start(
                out=outT[mt * 128:(mt + 1) * 128, qc * QC:(qc + 1) * QC],
                in_=ob,
            )

        # ---- emission schedule -----------------------------------------
        # Prologue: k c0 + q c0/c1 projections feed the first scores ASAP;
        # the rest of the k projection follows, then the attention stream
        # starts with remaining projections and out-proj(qc0) as fillers.
        proj_kq(wk_sb, bk_sb, kT_sb, 0, 0, "st0")
        proj_kq(wk_sb, bk_sb, kT_sb, 0, 1, "st1")
        proj_kq(wq_sb, bq_sb, qT_sb, 0, 0, "ot0")
        proj_kq(wq_sb, bq_sb, qT_sb, 0, 1, "ot1")
        proj_kq(wq_sb, bq_sb, qT_sb, 1, 0, "st0")
        proj_kq(wq_sb, bq_sb, qT_sb, 1, 1, "st1")
        for c in (1, 2, 3):
            proj_kq(wk_sb, bk_sb, kT_sb, c, 0, "ot0" if c == 1 else "st0")
            proj_kq(wk_sb, bk_sb, kT_sb, c, 1, "ot1" if c == 1 else "st1")
        proj_v(0, "ot0", nc.vector)
        proj_v(1, "ot1", nc.vector)

        # filler queue: list of emission thunks, popped between kt steps
        fillers = []
        for lt in range(2, NLT):
            fillers.append(("v", lt))
        for c in (2, 3):
            for jt in range(NJT):
                fillers.append(("q", c, jt))

        def emit_filler(tag, copy_rr):
            if not fillers:
                return
            f = fillers.pop(0)
            if f[0] == "v":
                proj_v(f[1], tag, nc.vector)
            elif f[0] == "q":
                proj_kq(wq_sb, bq_sb, qT_sb, f[1], f[2], tag)
            else:
                outproj_mt(f[1], f[2], tag, nc.vector)

        frr = [0]

        for qc in range(NQC):
            if qc == 1:
                for mt in range(NDT):
                    fillers.append(("o", 0, mt))
            for hp in range(NJT):
                ots = {h: psum.tile([HD + 1, QC], F32, tag=f"ot{h}",
                                    name=f"ot{h}")
                       for h in (0, 1)}
                for kt in range(NLT):
                    at0 = score_exp(hp, qc, kt, 0)
                    at1 = score_exp(hp, qc, kt, 1)
                    attnv(hp, kt, 0, at0, ots[0])
                    attnv(hp, kt, 1, at1, ots[1])
                    if kt % 2 == 1:
                        emit_filler(f"st{(kt // 2) % 2}", frr[0])
                        frr[0] += 1
                epilogue(hp, qc, ots)
        # tail: out-proj of the last q-chunk, deep pipeline over all tags
        for mt in range(NDT):
            outproj_mt(1, mt, ("st0", "st1", "ot0", "ot1")[mt % 4],
                       (nc.vector, nc.scalar)[mt % 2])

    nc.compile()
    return nc


_NC_CACHE = {}


def _get_nc():
    if "nc" not in _NC_CACHE:
        _NC_CACHE["nc"] = build_attention_core()
    return _NC_CACHE["nc"]


def _pack_x(x):
    """[L, D] fp32 -> [128, NDT, L] bf16 with d = t*128 + p."""
    xT = np.ascontiguousarray(np.asarray(x, np.float32).T)       # [D, L]
    return np.ascontiguousarray(
        xT.reshape(8, 128, xT.shape[1]).transpose(1, 0, 2)).astype(NPBF16)


def _pack_w(w):
    """[D, JC] fp32 -> [128, NDT, JC] bf16."""
    w = np.asarray(w, np.float32)
    return np.ascontiguousarray(
        w.reshape(8, 128, w.shape[1])).transpose(1, 0, 2).astype(NPBF16)


def shard_inputs(query, key_, value, Wq, bq, Wk, bk, Wv, bv, Wo, bo,
                 B=2, H=16, H_LOC=4, HD=64):
    """Host-side sharding: core c -> (batch c//4, head-group c%4)."""
    groups = H // H_LOC
    JC = H_LOC * HD
    xq = [_pack_x(np.asarray(query, np.float32)[b]) for b in range(B)]
    xk = [_pack_x(np.asarray(key_, np.float32)[b]) for b in range(B)]
    xv = [_pack_x(np.asarray(value, np.float32)[b]) for b in range(B)]
    in_maps = []
    for c in range(B * groups):
        b, g = divmod(c, groups)
        js = slice(g * JC, (g + 1) * JC)
        wo_s = np.asarray(Wo, np.float32)[js, :]                  # [JC, D]
        in_maps.append({
            "xq": xq[b], "xk": xk[b], "xv": xv[b],
            "wq": _pack_w(np.asarray(Wq, np.float32)[:, js]),
            "wk": _pack_w(np.asarray(Wk, np.float32)[:, js]),
            "wv": _pack_w(np.asarray(Wv, np.float32)[:, js]),
            "wo": np.ascontiguousarray(
                wo_s.reshape(2, 128, -1)).transpose(1, 0, 2).astype(NPBF16),
            "bq": np.ascontiguousarray(
                np.asarray(bq, np.float32)[js].reshape(2, 128).T),
            "bk": np.ascontiguousarray(
                np.asarray(bk, np.float32)[js].reshape(2, 128).T),
            "bv": np.ascontiguousarray(np.broadcast_to(
                np.asarray(bv, np.float32)[js], (128, JC))),
        })
    return in_maps


def kernel(query, key_, value, Wq, bq, Wk, bk, Wv, bv, Wo, bo):
    B, L, D = 2, 2048, 1024
    groups = 4
    nc = _get_nc()
    in_maps = shard_inputs(query, key_, value, Wq, bq, Wk, bk, Wv, bv, Wo, bo)
    res = run_bass_kernel_spmd(nc, in_maps, list(range(8))).results
    out = np.empty((B, L, D), np.float32)
    bo = np.asarray(bo, np.float32)
    for b in range(B):
        acc = res[b * groups]["outT"].astype(np.float32)
        for g in range(1, groups):
            acc = acc + res[b * groups + g]["outT"]
        out[b] = acc.T + bo
    return out


# revision 12
# speedup vs baseline: 1.0891x; 1.0891x over previous
"""Memory-efficient multi-head attention on 8 TRN2 NeuronCores (bf16 core).

Problem (hardcoded): B=2, L=2048, D=1024, H=16, HD=64.
  out = softmax((X_q Wq + bq)(X_k Wk + bk)^T / sqrt(HD)) (X_v Wv + bv) Wo + bo

Sharding: 8 cores = 2 batches x 4 head-groups (4 heads each).  Each core gets
its batch's activations (transposed, bf16) plus its head-group's weight
slices, and produces a partial pre-bias output outT [D, L] in fp32.  Host
sums the 4 partials per batch and adds bo.

Device kernel design (cost-model-driven):
  - All matmul operands are bf16 (1 PE cycle/row, same as fp32r, but half
    the HBM traffic and SBUF).  PSUM accumulation stays fp32.  bf16
    per-element rounding (~0.1% rms) contributes ~0.2% output error vs the
    2e-2 budget.  (fp8 DoubleRow was evaluated and measured ~9% error - the
    attention average shrinks signal as fast as noise, so per-element fp8
    error passes through at full strength.  Rejected.)
  - Scores are computed transposed, sT[kpos, q] = k . q, per head on disjoint
    64-partition groups; softmax denominator comes from an appended ones
    column on v (attnv accumulator row 64).
  - exp on ScalarE with the 1/sqrt(HD) scale folded in, no max-subtraction
    (scores ~N(0,1); softmax is shift-invariant; fp32/bf16 range is ample).
  - Biases are applied by DVE during PSUM->SBUF copy-out (tensor_scalar_add
    with per-partition bias for q/k, tensor_tensor add with a host-
    broadcast bias tile for v) - no PE bias matmuls.
  - A Pool-engine memset chain paces 6 tiny dummy matmuls over the first
    ~9us so the PE p-state ramp (cost model: warm after 3us of busy) is
    done before real matmuls arrive.
  - PE is the bottleneck (~164us busy); the emission order keeps it fed:
    prologue projections, then an ACT-paced score/exp/attnv stream with
    v-proj, q-proj(qc1) and out-proj(qc0) chunks interleaved as fillers in
    the PSUM slot gaps, out-proj(qc1) as the tail.
"""

import numpy as np
import ml_dtypes

import concourse.mybir as mybir
import concourse.tile as tile
from concourse import bacc
from concourse.bass_utils import run_bass_kernel_spmd

F32 = mybir.dt.float32
BF16 = mybir.dt.bfloat16
NPBF16 = ml_dtypes.bfloat16


def build_attention_core(L=2048, D=1024, H_LOC=4, HD=64):
    JC = H_LOC * HD                   # local head channels (256)
    NJT = JC // 128                   # j-tiles (2)
    NDT = D // 128                    # contraction tiles (8)
    NLT = L // 128                    # k-position tiles (16)
    XC = 512                          # x stream chunk width
    NXC = L // XC                     # 4
    QC = 1024                         # attention q-chunk (exp tile width)
    NQC = L // QC                     # 2
    CS = 512                          # matmul N-chunk inside a q-chunk
    NCS = QC // CS                    # 2

    nc = bacc.Bacc("TRN2", target_bir_lowering=False, debug=False, num_devices=8)

    xq = nc.dram_tensor("xq", [128, NDT, L], BF16, kind="ExternalInput").ap()
    xk = nc.dram_tensor("xk", [128, NDT, L], BF16, kind="ExternalInput").ap()
    xv = nc.dram_tensor("xv", [128, NDT, L], BF16, kind="ExternalInput").ap()
    wq = nc.dram_tensor("wq", [128, NDT, JC], BF16, kind="ExternalInput").ap()
    wk = nc.dram_tensor("wk", [128, NDT, JC], BF16, kind="ExternalInput").ap()
    wv = nc.dram_tensor("wv", [128, NDT, JC], BF16, kind="ExternalInput").ap()
    wo = nc.dram_tensor("wo", [128, NJT, D], BF16, kind="ExternalInput").ap()
    bq = nc.dram_tensor("bq", [128, NJT], F32, kind="ExternalInput").ap()
    bk = nc.dram_tensor("bk", [128, NJT], F32, kind="ExternalInput").ap()
    bv = nc.dram_tensor("bv", [128, JC], F32, kind="ExternalInput").ap()
    outT = nc.dram_tensor("outT", [D, L], F32, kind="ExternalOutput").ap()

    from contextlib import ExitStack

    with (
        tile.TileContext(nc) as tc,
        ExitStack() as stack,
        nc.allow_low_precision(reason="bf16 operands, fp32 accumulation"),
    ):
        consts = stack.enter_context(tc.tile_pool(name="consts", bufs=1))
        warm = stack.enter_context(tc.tile_pool(name="warm", bufs=1))
        wpool = stack.enter_context(tc.tile_pool(name="wpool", bufs=1))
        xpool = stack.enter_context(tc.tile_pool(name="xpool", bufs=1))
        prod = stack.enter_context(tc.tile_pool(name="prod", bufs=1))
        att_pool = stack.enter_context(tc.tile_pool(name="att", bufs=4))
        rden_pool = stack.enter_context(tc.tile_pool(name="rden", bufs=2))
        oc_pool = stack.enter_context(tc.tile_pool(name="oc", bufs=4))
        psum = stack.enter_context(tc.tile_pool(name="psum", bufs=1, space="PSUM"))

        # ---- PE warmup: Pool memset chain paces tiny matmuls ~1.5us apart
        wsrc = warm.tile([1, 8], BF16, tag="wsrc")
        nc.vector.memset(wsrc, 0.0)
        ps_wm = psum.tile([1, 8], F32, tag="st0", name="ps_wm")
        for i in range(6):
            wch = warm.tile([1, 1800], F32, tag="wch", name="wch")
            nc.gpsimd.memset(wch, 0.0)
            gate = warm.tile([1, 2], BF16, tag="wg", bufs=2, name="gate")
            nc.vector.tensor_copy(out=gate, in_=wch[:, 0:2])
            nc.tensor.matmul(ps_wm[:, 0:1], gate[:, 0:1], gate[:, 1:2],
                             start=True, stop=True)

        # ---- weights + biases (lead the DMA queue)
        wk_sb = wpool.tile([128, NDT, JC], BF16, tag="wk")
        wq_sb = wpool.tile([128, NDT, JC], BF16, tag="wq")
        wv_sb = wpool.tile([128, NDT, JC], BF16, tag="wv")
        wo_sb = wpool.tile([128, NJT, D], BF16, tag="wo")
        bq_sb = wpool.tile([128, NJT], F32, tag="bq")
        bk_sb = wpool.tile([128, NJT], F32, tag="bk")
        bv_sb = wpool.tile([128, JC], F32, tag="bv")
        nc.sync.dma_start(out=wk_sb, in_=wk)
        nc.sync.dma_start(out=bk_sb, in_=bk)
        nc.sync.dma_start(out=wq_sb, in_=wq)
        nc.sync.dma_start(out=bq_sb, in_=bq)

        # ---- x streams (SBUF-resident, chunked DMA)
        xk_sb = xpool.tile([128, NDT, L], BF16, tag="xk")
        xq_sb = xpool.tile([128, NDT, L], BF16, tag="xq")
        xv_sb = xpool.tile([128, NDT, L], BF16, tag="xv")

        def load_x(eng, x_sb, x_dram, c):
            eng.dma_start(out=x_sb[:, :, c * XC:(c + 1) * XC],
                          in_=x_dram[:, :, c * XC:(c + 1) * XC])

        # All loads on ONE queue (SP) in priority order: the DMA_ENGINES
        # device serializes transfers, so a second queue would let late
        # loads jump ahead of the critical prefix.
        load_x(nc.sync, xk_sb, xk, 0)
        load_x(nc.sync, xq_sb, xq, 0)
        load_x(nc.sync, xq_sb, xq, 1)
        nc.sync.dma_start(out=wv_sb, in_=wv)
        nc.sync.dma_start(out=bv_sb, in_=bv)
        load_x(nc.sync, xk_sb, xk, 1)
        load_x(nc.sync, xv_sb, xv, 0)
        load_x(nc.sync, xk_sb, xk, 2)
        load_x(nc.sync, xv_sb, xv, 1)
        load_x(nc.sync, xk_sb, xk, 3)
        load_x(nc.sync, xv_sb, xv, 2)
        load_x(nc.sync, xv_sb, xv, 3)
        load_x(nc.sync, xq_sb, xq, 2)
        load_x(nc.sync, xq_sb, xq, 3)
        nc.sync.dma_start(out=wo_sb, in_=wo)

        # ---- products
        qT_sb = prod.tile([128, NJT, L], BF16, tag="qT")    # [j%128, jt, l]
        kT_sb = prod.tile([128, NJT, L], BF16, tag="kT")
        v_sb = prod.tile([128, NLT, H_LOC, HD + 1], BF16, tag="v")
        onorm_sb = prod.tile([128, NJT, L], BF16, tag="onorm")

        vones_f32 = consts.tile([128, NLT * H_LOC], F32)
        nc.vector.memset(vones_f32, 1.0)
        nc.vector.tensor_copy(
            out=v_sb[:, :, :, HD:HD + 1].rearrange("p a b c -> p (a b c)"),
            in_=vones_f32,
        )

        # ---- emission helpers ------------------------------------------
        def proj_kq(w_sb, b_sb, dst, c, jt, tag):
            """dst[j, l] = sum_d W[d, j] xT[d, l] + b[j] for one (chunk, jt)."""
            x_sb = xk_sb if dst is kT_sb else xq_sb
            ps = psum.tile([128, XC], F32, tag=tag, name="pskq")
            for dt in range(NDT):
                nc.tensor.matmul(
                    ps,
                    w_sb[:, dt, jt * 128:(jt + 1) * 128],
                    x_sb[:, dt, c * XC:(c + 1) * XC],
                    start=(dt == 0),
                    stop=(dt == NDT - 1),
                )
            nc.vector.tensor_scalar_add(
                out=dst[:, jt, c * XC:(c + 1) * XC],
                in0=ps,
                scalar1=b_sb[:, jt:jt + 1],
            )

        def proj_v(lt, tag, copy_eng):
            """v[l, (h, hd)] = sum_d xT[d, l] W[d, j] + bv[j], one 128-row tile."""
            ps = psum.tile([128, JC], F32, tag=tag, name="psv")
            for dt in range(NDT):
                nc.tensor.matmul(
                    ps,
                    xv_sb[:, dt, lt * 128:lt * 128 + 128],
                    wv_sb[:, dt, :],
                    start=(dt == 0),
                    stop=(dt == NDT - 1),
                )
            copy_eng.tensor_tensor(
                out=v_sb[:, lt, :, 0:HD],
                in0=ps.rearrange("p (h d) -> p h d", h=H_LOC),
                in1=bv_sb.rearrange("p (h d) -> p h d", h=H_LOC),
                op=mybir.AluOpType.add,
            )

        def score_exp(hp, qc, kt, h):
            """One head's transposed scores + exp at one kt.  Returns at tile."""
            hb = (h % 2) * HD
            st = psum.tile([128, QC], F32, tag=f"st{h % 2}", name=f"st{h % 2}")
            for c in range(NCS):
                nc.tensor.matmul(
                    st[:, c * CS:(c + 1) * CS],
                    kT_sb[hb:hb + HD, hp, kt * 128:kt * 128 + 128],
                    qT_sb[hb:hb + HD, hp,
                          qc * QC + c * CS:qc * QC + (c + 1) * CS],
                    start=True,
                    stop=True,
                )
            at = att_pool.tile([128, QC], BF16, tag=f"at{h % 2}",
                               name=f"at{h % 2}")
            nc.scalar.activation(
                out=at, in_=st,
                func=mybir.ActivationFunctionType.Exp,
                scale=float(1.0 / np.sqrt(HD)),
            )
            return at

        def attnv(hp, kt, h, at, ot):
            hg = 2 * hp + (h % 2)
            for c in range(NCS):
                nc.tensor.matmul(
                    ot[:, c * CS:(c + 1) * CS],
                    v_sb[:, kt, hg, :],
                    at[:, c * CS:(c + 1) * CS],
                    start=(kt == 0),
                    stop=(kt == NLT - 1),
                )

        def epilogue(hp, qc, ots, chunks=1):
            """onorm[ch, q] = OT[ch, q] * recip(OT[HD, q])."""
            W = QC // chunks
            for cc in range(chunks):
                for h in (0, 1):
                    hb = h * HD
                    rden = rden_pool.tile([1, W], F32, tag="rden", name="rden")
                    nc.vector.reciprocal(
                        out=rden, in_=ots[h][HD:HD + 1, cc * W:(cc + 1) * W])
                    rbc = rden_pool.tile([HD, W], F32, tag="rbc", name="rbc")
                    nc.gpsimd.partition_broadcast(rbc, rden)
                    nc.vector.tensor_mul(
                        out=onorm_sb[hb:hb + HD, hp,
                                     qc * QC + cc * W:qc * QC + (cc + 1) * W],
                        in0=ots[h][0:HD, cc * W:(cc + 1) * W],
                        in1=rbc,
                    )

        _store_rr = [0]

        def outproj_mt(qc, mt, tag, copy_eng):
            """outT[dp, l] = sum_j wo[j, dp] onorm[j, l], one 128-row tile."""
            ps = psum.tile([128, QC], F32, tag=tag, name="psc")
            for c in range(NCS):
                for jt in range(NJT):
                    nc.tensor.matmul(
                        ps[:, c * CS:(c + 1) * CS],
                        wo_sb[:, jt, mt * 128:(mt + 1) * 128],
                        onorm_sb[:, jt,
                                 qc * QC + c * CS:qc * QC + (c + 1) * CS],
                        start=(jt == 0),
                        stop=(jt == NJT - 1),
                    )
            ob = oc_pool.tile([128, QC], F32, tag="oc", name="oc")
            if copy_eng is nc.scalar:
                nc.scalar.copy(out=ob, in_=ps)
            else:
                copy_eng.tensor_copy(out=ob, in_=ps)
            dma_eng = (nc.sync, nc.scalar)[_store_rr[0] % 2]
            _store_rr[0] += 1
            dma_eng.dma_start(
                out=outT[mt * 128:(mt + 1) * 128, qc * QC:(qc + 1) * QC],
                in_=ob,
            )

        # ---- emission schedule -----------------------------------------
        # Minimal prologue (k c0 + q c0/c1 + v lt0 feed the first steps),
        # then the ACT-paced score/exp/attnv stream with every remaining
        # projection and out-proj(qc0) emitted as deadline-driven fillers
        # inside the stream, borrowing st PSUM slots.
        proj_kq(wk_sb, bk_sb, kT_sb, 0, 0, "st0")
        proj_kq(wk_sb, bk_sb, kT_sb, 0, 1, "st1")
        proj_kq(wq_sb, bq_sb, qT_sb, 0, 0, "ot0")
        proj_kq(wq_sb, bq_sb, qT_sb, 0, 1, "ot1")
        proj_kq(wq_sb, bq_sb, qT_sb, 1, 0, "st0")
        proj_kq(wq_sb, bq_sb, qT_sb, 1, 1, "st1")
        proj_v(0, "ot0", nc.vector)

        # (step, thunk) deadline-ordered filler queue for the global step
        # counter (64 kt-steps across qc/hp).  kproj chunk c feeds scores
        # from kt=4c; vproj lt feeds attnv at kt=lt; qproj c2/c3 feed qc1.
        fillers = []
        for c in (1, 2, 3):
            for jt in range(NJT):
                fillers.append((4 * c - 3 + jt, ("k", c, jt)))
        for lt in range(1, NLT):
            fillers.append((lt - 1, ("v", lt)))
        for c, jt in (((2, 0)), (2, 1), (3, 0), (3, 1)):
            fillers.append((18 + 2 * (2 * (c - 2) + jt), ("q", c, jt)))
        for mt in range(NDT):
            fillers.append((33 + 3 * mt, ("o", 0, mt)))
        fillers.sort(key=lambda x: x[0])

        _frr = [0]

        def emit_fillers(step, last=False):
            while fillers and (fillers[0][0] <= step or last):
                _, f = fillers.pop(0)
                tag = f"st{_frr[0] % 2}"
                _frr[0] += 1
                if f[0] == "v":
                    proj_v(f[1], tag, nc.vector)
                elif f[0] == "k":
                    proj_kq(wk_sb, bk_sb, kT_sb, f[1], f[2], tag)
                elif f[0] == "q":
                    proj_kq(wq_sb, bq_sb, qT_sb, f[1], f[2], tag)
                else:
                    outproj_mt(f[1], f[2], tag, nc.vector)

        step = 0
        for qc in range(NQC):
            for hp in range(NJT):
                ots = {h: psum.tile([HD + 1, QC], F32, tag=f"ot{h}",
                                    name=f"ot{h}")
                       for h in (0, 1)}
                for kt in range(NLT):
                    at0 = score_exp(hp, qc, kt, 0)
                    at1 = score_exp(hp, qc, kt, 1)
                    attnv(hp, kt, 0, at0, ots[0])
                    attnv(hp, kt, 1, at1, ots[1])
                    # keep the last two out-proj(qc0) tiles in reserve to
                    # bridge the PE gap over the final epilogue
                    if not (qc == 1 and hp == 1 and len(fillers) <= 2):
                        emit_fillers(step)
                    step += 1
                epilogue(hp, qc, ots,
                         chunks=2 if (qc == 1 and hp == 1) else 1)
                if qc == 1 and hp == 1:
                    emit_fillers(step, last=True)
        # tail: out-proj of the last q-chunk, deep pipeline over all tags
        for mt in range(NDT):
            outproj_mt(1, mt, ("st0", "st1", "ot0", "ot1")[mt % 4],
                       (nc.vector, nc.scalar)[mt % 2])

    nc.compile()
    return nc


_NC_CACHE = {}


def _get_nc():
    if "nc" not in _NC_CACHE:
        _NC_CACHE["nc"] = build_attention_core()
    return _NC_CACHE["nc"]


def _pack_x(x):
    """[L, D] fp32 -> [128, NDT, L] bf16 with d = t*128 + p."""
    xT = np.ascontiguousarray(np.asarray(x, np.float32).T)       # [D, L]
    return np.ascontiguousarray(
        xT.reshape(8, 128, xT.shape[1]).transpose(1, 0, 2)).astype(NPBF16)


def _pack_w(w):
    """[D, JC] fp32 -> [128, NDT, JC] bf16."""
    w = np.asarray(w, np.float32)
    return np.ascontiguousarray(
        w.reshape(8, 128, w.shape[1])).transpose(1, 0, 2).astype(NPBF16)


def shard_inputs(query, key_, value, Wq, bq, Wk, bk, Wv, bv, Wo, bo,
                 B=2, H=16, H_LOC=4, HD=64):
    """Host-side sharding: core c -> (batch c//4, head-group c%4)."""
    groups = H // H_LOC
    JC = H_LOC * HD
    xq = [_pack_x(np.asarray(query, np.float32)[b]) for b in range(B)]
    xk = [_pack_x(np.asarray(key_, np.float32)[b]) for b in range(B)]
    xv = [_pack_x(np.asarray(value, np.float32)[b]) for b in range(B)]
    in_maps = []
    for c in range(B * groups):
        b, g = divmod(c, groups)
        js = slice(g * JC, (g + 1) * JC)
        wo_s = np.asarray(Wo, np.float32)[js, :]                  # [JC, D]
        in_maps.append({
            "xq": xq[b], "xk": xk[b], "xv": xv[b],
            "wq": _pack_w(np.asarray(Wq, np.float32)[:, js]),
            "wk": _pack_w(np.asarray(Wk, np.float32)[:, js]),
            "wv": _pack_w(np.asarray(Wv, np.float32)[:, js]),
            "wo": np.ascontiguousarray(
                wo_s.reshape(2, 128, -1)).transpose(1, 0, 2).astype(NPBF16),
            "bq": np.ascontiguousarray(
                np.asarray(bq, np.float32)[js].reshape(2, 128).T),
            "bk": np.ascontiguousarray(
                np.asarray(bk, np.float32)[js].reshape(2, 128).T),
            "bv": np.ascontiguousarray(np.broadcast_to(
                np.asarray(bv, np.float32)[js], (128, JC))),
        })
    return in_maps


def kernel(query, key_, value, Wq, bq, Wk, bk, Wv, bv, Wo, bo):
    B, L, D = 2, 2048, 1024
    groups = 4
    nc = _get_nc()
    in_maps = shard_inputs(query, key_, value, Wq, bq, Wk, bk, Wv, bv, Wo, bo)
    res = run_bass_kernel_spmd(nc, in_maps, list(range(8))).results
    out = np.empty((B, L, D), np.float32)
    bo = np.asarray(bo, np.float32)
    for b in range(B):
        acc = res[b * groups]["outT"].astype(np.float32)
        for g in range(1, groups):
            acc = acc + res[b * groups + g]["outT"]
        out[b] = acc.T + bo
    return out
